# revision 6
# baseline (speedup 1.0000x reference)
"""Trainium2 Bass kernel for the batched constant-velocity Kalman filter.

Math: the 4-state (x, y, vx, vy) filter with H = [[1,0,0,0],[0,1,0,0]],
diagonal Q/R/P0 decouples into two identical 2-state (pos, vel) filters
sharing a single Riccati recursion (the covariance depends only on the
mask sequence, which is shared by the x and y axes).

Carried per filter (4096 filters per core = 128 partitions x 32 lanes):
  a = pp00, b = pp01, c = pp11   (predicted covariance, shared x/y)
  u = predicted positions (x, y), v = predicted velocities (x, y)

Per step (m = mask in {0,1}):
  r  = 1/(a+1);  rm = r*m
  k0m = a*rm; k1m = b*rm            (masked gains)
  g  = 1 - k0m
  w  = c - b*k1m                     (= updated p11)
  a' = a*g + 2*b*g + w + 0.01
  b' = b*g + w
  c' = w + 0.01
  y  = z - u
  u' = (u + v) + (k0m + k1m)*y
  v' = v + k1m*y
Final output (after last update, no re-predict): pos = u + k0m*y.

Sharding: data-parallel over the batch axis, 16 batches per core x 8 cores.
"""

import numpy as np

import concourse.bass as bass
import concourse.tile as tile
from concourse import bacc, mybir

F32 = mybir.dt.float32
AF = mybir.ActivationFunctionType

B, T, V = 128, 256, 256
NCORES = 8
BT = B // NCORES          # batches per core
W = 32                    # lane stripe width (vehicles per partition)
P = 128                   # partitions
CH = 8                    # time steps per input DMA chunk

A0, B0, C0 = 2000.01, 1000.0, 1000.01  # F P0 F^T + Q entries


def _ap(base, doff, dims):
    """AP with base's partition dim, extra element offset, custom free dims."""
    return bass.AP(
        tensor=base.tensor,
        offset=base.offset + doff,
        ap=[list(base.ap[0])] + [[s, c] for s, c in dims],
    )


def _emit(tc, out_ap, in_ap, Tsteps):
    nc = tc.nc
    import contextlib

    ctx = contextlib.ExitStack()
    with ctx:
        eng_p = nc.vector    # Riccati chain engine
        eng_s = nc.gpsimd    # state-update engine
        eng_a = nc.scalar    # affine helper engine (ACT)

        nch = (Tsteps + CH - 1) // CH

        in_pool = ctx.enter_context(tc.tile_pool(name="inp", bufs=4))
        m_pool = ctx.enter_context(tc.tile_pool(name="mask", bufs=4))
        ab_pool = ctx.enter_context(tc.tile_pool(name="ab", bufs=2))
        gpc_pool = ctx.enter_context(tc.tile_pool(name="gpc", bufs=2))
        kkt_pool = ctx.enter_context(tc.tile_pool(name="kkt", bufs=2))
        uv_pool = ctx.enter_context(tc.tile_pool(name="uv", bufs=2))
        scr_pool = ctx.enter_context(tc.tile_pool(name="scr", bufs=2))
        const_pool = ctx.enter_context(tc.tile_pool(name="const", bufs=1))

        ones = const_pool.tile([P, W], F32)
        nc.vector.memset(ones[:], 1.0)
        out_t = const_pool.tile([P, W, 3], F32)   # (1, px, py) interleaved
        nc.vector.memset(_ap(out_t[:], 0, [(3, W)]), 1.0)
        cq = const_pool.tile([P, 1], F32)      # +0.01 bias
        nc.vector.memset(cq[:], 0.01)
        cqn = const_pool.tile([P, 1], F32)     # -0.01 bias
        nc.vector.memset(cqn[:], -0.01)

        # initial carried state
        ab = ab_pool.tile([P, 2, W], F32)       # [a | b]
        nc.vector.memset(ab[:, 0, :], A0)
        nc.vector.memset(ab[:, 1, :], B0)
        gpc = gpc_pool.tile([P, 3, W], F32)     # [ag | bg | c]
        nc.vector.memset(gpc[:, 2, :], C0)
        uv = uv_pool.tile([P, 6, W], F32)       # [t4x t4y | ux uy | vx vy]
        nc.vector.memset(uv[:], 0.0)

        in_tiles = [None] * nch
        m_tiles = [None] * nch

        def load_chunk(c):
            t0 = c * CH
            csz = min(CH, Tsteps - t0)
            it = in_pool.tile([P, CH, 3 * W], F32)
            for b in range(BT):
                src = in_ap[b, t0:t0 + csz].rearrange(
                    "t (vb vj) c -> vb t (vj c)", vb=P // BT
                )
                nc.sync.dma_start(out=it[b * 8:(b + 1) * 8, :csz, :], in_=src)
            mt = m_pool.tile([P, CH, W], F32)
            # m = 0.5*L + 0.5 from the label plane (stride-3)
            lbl = _ap(it[:], 0, [(3 * W, csz), (3, W)])
            nc.scalar.activation(mt[:, :csz, :], lbl, AF.Copy, bias=0.5, scale=0.5)
            in_tiles[c] = it
            m_tiles[c] = mt

        load_chunk(0)
        if nch > 1:
            load_chunk(1)

        for t in range(Tsteps):
            c, s = divmod(t, CH)
            if s == 0 and c + 2 < nch:
                load_chunk(c + 2)
            it = in_tiles[c]
            mt = m_tiles[c]
            m_s = mt[:, s, :]
            z_s = _ap(it[:], s * 3 * W + 1, [(1, 2), (3, W)])  # [z0 | z1]

            last = t == Tsteps - 1

            # ---- Riccati chain (eng_p) ----
            e = scr_pool.tile([P, W], F32)
            eng_p.tensor_scalar_add(e[:], ab[:, 0, :], 1.0)
            r = scr_pool.tile([P, W], F32)
            eng_p.reciprocal(r[:], e[:])
            rm = scr_pool.tile([P, W], F32)
            eng_p.tensor_mul(rm[:], r[:], m_s)
            kkt = kkt_pool.tile([P, 3, W], F32)  # [k0m | kks | k1m]
            eng_p.tensor_mul(
                kkt[:, 0::2, :], ab[:], _ap(rm[:], 0, [(0, 2), (1, W)])
            )

            # ---- state update (eng_s) ----
            y = scr_pool.tile([P, 2, W], F32)
            eng_s.tensor_sub(y[:], z_s, uv[:, 2:4, :])

            if not last:
                # gains sum for the u-update
                eng_p.tensor_add(kkt[:, 1, :], kkt[:, 0, :], kkt[:, 2, :])
                # g = 1 - k0m
                g = scr_pool.tile([P, W], F32)
                eng_p.scalar_tensor_tensor(
                    g[:], kkt[:, 0, :], -1.0, ones[:],
                    mybir.AluOpType.mult, mybir.AluOpType.add,
                )
                # t_ = b*k1m ; w = c - t_
                t_ = scr_pool.tile([P, W], F32)
                eng_s.tensor_mul(t_[:], ab[:, 1, :], kkt[:, 2, :])
                w = scr_pool.tile([P, W], F32)
                eng_s.tensor_sub(w[:], gpc[:, 2, :], t_[:])
                # [ag | bg] and next c
                gpc_n = gpc_pool.tile([P, 3, W], F32)
                eng_p.tensor_mul(
                    gpc_n[:, 0:2, :], ab[:], _ap(g[:], 0, [(0, 2), (1, W)])
                )
                eng_a.activation(gpc_n[:, 2, :], w[:], AF.Identity, bias=cq[:])  # c' = w + 0.01
                # [x1 | q2] = [ag | bg] + [bg | c']
                x1q2 = scr_pool.tile([P, 2, W], F32)
                eng_p.tensor_add(x1q2[:], gpc_n[:, 0:2, :], gpc_n[:, 1:3, :])
                ab_n = ab_pool.tile([P, 2, W], F32)
                eng_p.tensor_add(ab_n[:, 0, :], x1q2[:, 0, :], x1q2[:, 1, :])
                eng_a.activation(ab_n[:, 1, :], x1q2[:, 1, :], AF.Identity, bias=cqn[:])  # b' = q2 - 0.01

                # q = [y|y] * [kks|k1m]  -> [q0x q0y | q1x q1y]
                q = scr_pool.tile([P, 2, 2, W], F32)
                eng_s.tensor_mul(
                    q[:],
                    _ap(y[:], 0, [(0, 2), (W, 2), (1, W)]),
                    _ap(kkt[:], W, [(W, 2), (0, 2), (1, W)]),
                )
                # t4 = u + v (into prev uv tile's scratch slots)
                eng_s.tensor_add(uv[:, 0:2, :], uv[:, 2:4, :], uv[:, 4:6, :])
                # [u' | v'] = [t4 | v] + q
                uv_n = uv_pool.tile([P, 6, W], F32)
                eng_s.tensor_add(
                    _ap(uv_n[:], 2 * W, [(2 * W, 2), (W, 2), (1, W)]),
                    _ap(uv[:], 0, [(4 * W, 2), (W, 2), (1, W)]),
                    q[:],
                )
                ab = ab_n
                gpc = gpc_n
                uv = uv_n
            else:
                # final: pos = u + k0m*y
                qf = scr_pool.tile([P, 2, W], F32)
                eng_s.tensor_mul(
                    qf[:], y[:], _ap(kkt[:], 0, [(0, 2), (1, W)])
                )
                eng_s.tensor_add(
                    _ap(out_t[:], 1, [(1, 2), (3, W)]), uv[:, 2:4, :], qf[:]
                )

        for b in range(BT):
            # out[b] is [256, 3] = (vb, vj, c); out_t is [P, W, 3] interleaved
            dst = bass.AP(
                tensor=out_ap.tensor,
                offset=out_ap.offset + b * V * 3,
                ap=[[3 * W, P // BT], [3, W], [1, 3]],
            )
            nc.sync.dma_start(out=dst, in_=out_t[b * 8:(b + 1) * 8, :, :])


_CACHE = {}


def build(Tsteps=T):
    if Tsteps in _CACHE:
        return _CACHE[Tsteps]
    nc = bacc.Bacc(
        "TRN2", target_bir_lowering=False, debug=False, enable_asserts=False
    )
    xin = nc.dram_tensor("batch", [BT, Tsteps, V, 3], F32, kind="ExternalInput")
    out = nc.dram_tensor("out", [BT, V, 3], F32, kind="ExternalOutput")
    with tile.TileContext(nc) as tc:
        _emit(tc, out.ap(), xin.ap(), Tsteps)
    nc.compile()
    _CACHE[Tsteps] = nc
    return nc


def kernel(batch: np.ndarray) -> np.ndarray:
    from concourse.bass_utils import run_bass_kernel_spmd

    batch = np.ascontiguousarray(batch, dtype=np.float32)
    b, t, v, c = batch.shape
    nc = build(t)
    per = b // NCORES
    in_maps = [
        {"batch": np.ascontiguousarray(batch[i * per:(i + 1) * per])}
        for i in range(NCORES)
    ]
    res = run_bass_kernel_spmd(nc, in_maps, core_ids=list(range(NCORES)))
    return np.concatenate([r["out"] for r in res.results], axis=0)


# revision 7
# speedup vs baseline: 1.1091x; 1.1091x over previous
"""Trainium2 Bass kernel for the batched constant-velocity Kalman filter.

Math: the 4-state (x, y, vx, vy) filter with H = [[1,0,0,0],[0,1,0,0]],
diagonal Q/R/P0 decouples into two identical 2-state (pos, vel) filters
sharing a single Riccati recursion (the covariance depends only on the
mask sequence, which is shared by the x and y axes).

Carried per filter (4096 filters per core = 128 partitions x 32 lanes):
  a = pp00, b = pp01, c = pp11   (predicted covariance, shared x/y)
  u = predicted positions (x, y), v = predicted velocities (x, y)

Per step (m = mask in {0,1}):
  r  = 1/(a+1);  rm = r*m
  k0m = a*rm; k1m = b*rm            (masked gains)
  g  = 1 - k0m
  w  = c - b*k1m                     (= updated p11)
  a' = a*g + 2*b*g + w + 0.01
  b' = b*g + w
  c' = w + 0.01
  y  = z - u
  u' = (u + v) + (k0m + k1m)*y
  v' = v + k1m*y
Final output (after last update, no re-predict): pos = u + k0m*y.

Sharding: data-parallel over the batch axis, 16 batches per core x 8 cores.
"""

import numpy as np

import concourse.bass as bass
import concourse.tile as tile
from concourse import bacc, mybir

F32 = mybir.dt.float32
AF = mybir.ActivationFunctionType

B, T, V = 128, 256, 256
NCORES = 8
BT = B // NCORES          # batches per core
W = 32                    # lane stripe width (vehicles per partition)
P = 128                   # partitions
CH = 8                    # time steps per input DMA chunk

A0, B0, C0 = 2000.01, 1000.0, 1000.01  # F P0 F^T + Q entries


def _ap(base, doff, dims):
    """AP with base's partition dim, extra element offset, custom free dims."""
    return bass.AP(
        tensor=base.tensor,
        offset=base.offset + doff,
        ap=[list(base.ap[0])] + [[s, c] for s, c in dims],
    )


def _emit(tc, out_ap, in_ap, Tsteps):
    nc = tc.nc
    import contextlib

    ctx = contextlib.ExitStack()
    with ctx:
        eng_p = nc.vector    # Riccati chain engine
        eng_s = nc.gpsimd    # state-update engine
        eng_a = nc.scalar    # affine helper engine (ACT)

        nch = (Tsteps + CH - 1) // CH

        in_pool = ctx.enter_context(tc.tile_pool(name="inp", bufs=4))
        m_pool = ctx.enter_context(tc.tile_pool(name="mask", bufs=4))
        ab_pool = ctx.enter_context(tc.tile_pool(name="ab", bufs=2))
        gpc_pool = ctx.enter_context(tc.tile_pool(name="gpc", bufs=2))
        kkt_pool = ctx.enter_context(tc.tile_pool(name="kkt", bufs=2))
        uv_pool = ctx.enter_context(tc.tile_pool(name="uv", bufs=2))
        scr_pool = ctx.enter_context(tc.tile_pool(name="scr", bufs=2))
        const_pool = ctx.enter_context(tc.tile_pool(name="const", bufs=1))

        ones = const_pool.tile([P, W], F32)
        nc.vector.memset(ones[:], 1.0)
        out_t = const_pool.tile([P, W, 3], F32)   # (1, px, py) interleaved
        nc.vector.memset(_ap(out_t[:], 0, [(3, W)]), 1.0)

        # initial carried state
        ab = ab_pool.tile([P, 3, W], F32)       # [a | b | b^2]
        nc.vector.memset(ab[:, 0, :], A0)
        nc.vector.memset(ab[:, 1, :], B0)
        nc.vector.memset(ab[:, 2, :], B0 * B0)
        gpc = gpc_pool.tile([P, 3, W], F32)     # [ag | bg | w]
        nc.vector.memset(gpc[:, 2, :], C0 - 0.01)
        uv = uv_pool.tile([P, 6, W], F32)       # [t4x t4y | ux uy | vx vy]
        nc.vector.memset(uv[:], 0.0)

        in_tiles = [None] * nch
        m_tiles = [None] * nch

        def load_chunk(c):
            t0 = c * CH
            csz = min(CH, Tsteps - t0)
            it = in_pool.tile([P, CH, 3 * W], F32)
            for b in range(BT):
                src = in_ap[b, t0:t0 + csz].rearrange(
                    "t (vb vj) c -> vb t (vj c)", vb=P // BT
                )
                nc.sync.dma_start(out=it[b * 8:(b + 1) * 8, :csz, :], in_=src)
            mt = m_pool.tile([P, CH, W], F32)
            # m = 0.5*L + 0.5 from the label plane (stride-3)
            lbl = _ap(it[:], 0, [(3 * W, csz), (3, W)])
            nc.scalar.activation(mt[:, :csz, :], lbl, AF.Copy, bias=0.5, scale=0.5)
            in_tiles[c] = it
            m_tiles[c] = mt

        load_chunk(0)
        if nch > 1:
            load_chunk(1)

        for t in range(Tsteps):
            c, s = divmod(t, CH)
            if s == 0 and c + 2 < nch:
                load_chunk(c + 2)
            it = in_tiles[c]
            mt = m_tiles[c]
            m_s = mt[:, s, :]
            z_s = _ap(it[:], s * 3 * W + 1, [(1, 2), (3, W)])  # [z0 | z1]

            last = t == Tsteps - 1

            # ---- Riccati chain (eng_p) ----
            e = scr_pool.tile([P, W], F32)
            eng_p.tensor_scalar_add(e[:], ab[:, 0, :], 1.0)
            r = scr_pool.tile([P, W], F32)
            eng_p.reciprocal(r[:], e[:])
            rm = scr_pool.tile([P, W], F32)
            eng_p.tensor_mul(rm[:], r[:], m_s)
            kkt = kkt_pool.tile([P, 4, W], F32)  # [kks | k0m | k1m | t_]
            eng_p.tensor_mul(
                kkt[:, 1:4, :], ab[:], _ap(rm[:], 0, [(0, 3), (1, W)])
            )

            # ---- state update (eng_s) ----
            y = scr_pool.tile([P, 2, W], F32)
            eng_s.tensor_sub(y[:], z_s, uv[:, 2:4, :])

            if not last:
                # g = 1 - k0m
                g = scr_pool.tile([P, W], F32)
                eng_p.scalar_tensor_tensor(
                    g[:], kkt[:, 1, :], -1.0, ones[:],
                    mybir.AluOpType.mult, mybir.AluOpType.add,
                )
                # [ag | bg]
                gpc_n = gpc_pool.tile([P, 3, W], F32)
                eng_p.tensor_mul(
                    gpc_n[:, 0:2, :], ab[:, 0:2, :], _ap(g[:], 0, [(0, 2), (1, W)])
                )
                # w' = (w + 0.01) - t_   (carried in gpc slot 2)
                eng_p.scalar_tensor_tensor(
                    gpc_n[:, 2, :], gpc[:, 2, :], 0.01, kkt[:, 3, :],
                    mybir.AluOpType.add, mybir.AluOpType.subtract,
                )
                # [x1 | q2] = [ag | bg] + [bg | w']
                x1q2 = scr_pool.tile([P, 2, W], F32)
                eng_p.tensor_add(x1q2[:], gpc_n[:, 0:2, :], gpc_n[:, 1:3, :])
                ab_n = ab_pool.tile([P, 3, W], F32)
                # a' = (x1 + 0.01) + q2
                eng_p.scalar_tensor_tensor(
                    ab_n[:, 0, :], x1q2[:, 0, :], 0.01, x1q2[:, 1, :],
                    mybir.AluOpType.add, mybir.AluOpType.add,
                )
                eng_a.copy(ab_n[:, 1, :], x1q2[:, 1, :])          # b' = q2
                eng_a.square(ab_n[:, 2, :], x1q2[:, 1, :])        # b'^2

                # kks = k0m + k1m (into kkt slot 0)
                eng_s.tensor_add(kkt[:, 0, :], kkt[:, 1, :], kkt[:, 2, :])
                # q = [y|y] * [kks|k1m]  -> [q0x q0y | q1x q1y]
                q = scr_pool.tile([P, 2, 2, W], F32)
                eng_s.tensor_mul(
                    q[:],
                    _ap(y[:], 0, [(0, 2), (W, 2), (1, W)]),
                    _ap(kkt[:], 0, [(2 * W, 2), (0, 2), (1, W)]),
                )
                # t4 = u + v (into prev uv tile's scratch slots)
                eng_s.tensor_add(uv[:, 0:2, :], uv[:, 2:4, :], uv[:, 4:6, :])
                # [u' | v'] = [t4 | v] + q
                uv_n = uv_pool.tile([P, 6, W], F32)
                eng_s.tensor_add(
                    _ap(uv_n[:], 2 * W, [(2 * W, 2), (W, 2), (1, W)]),
                    _ap(uv[:], 0, [(4 * W, 2), (W, 2), (1, W)]),
                    q[:],
                )
                ab = ab_n
                gpc = gpc_n
                uv = uv_n
            else:
                # final: pos = u + k0m*y
                qf = scr_pool.tile([P, 2, W], F32)
                eng_s.tensor_mul(
                    qf[:], y[:], _ap(kkt[:], W, [(0, 2), (1, W)])
                )
                eng_s.tensor_add(
                    _ap(out_t[:], 1, [(1, 2), (3, W)]), uv[:, 2:4, :], qf[:]
                )

        for b in range(BT):
            # out[b] is [256, 3] = (vb, vj, c); out_t is [P, W, 3] interleaved
            dst = bass.AP(
                tensor=out_ap.tensor,
                offset=out_ap.offset + b * V * 3,
                ap=[[3 * W, P // BT], [3, W], [1, 3]],
            )
            nc.sync.dma_start(out=dst, in_=out_t[b * 8:(b + 1) * 8, :, :])


_CACHE = {}


def build(Tsteps=T):
    if Tsteps in _CACHE:
        return _CACHE[Tsteps]
    nc = bacc.Bacc(
        "TRN2", target_bir_lowering=False, debug=False, enable_asserts=False
    )
    xin = nc.dram_tensor("batch", [BT, Tsteps, V, 3], F32, kind="ExternalInput")
    out = nc.dram_tensor("out", [BT, V, 3], F32, kind="ExternalOutput")
    with tile.TileContext(nc) as tc:
        _emit(tc, out.ap(), xin.ap(), Tsteps)
    nc.compile()
    _CACHE[Tsteps] = nc
    return nc


def kernel(batch: np.ndarray) -> np.ndarray:
    from concourse.bass_utils import run_bass_kernel_spmd

    batch = np.ascontiguousarray(batch, dtype=np.float32)
    b, t, v, c = batch.shape
    nc = build(t)
    per = b // NCORES
    in_maps = [
        {"batch": np.ascontiguousarray(batch[i * per:(i + 1) * per])}
        for i in range(NCORES)
    ]
    res = run_bass_kernel_spmd(nc, in_maps, core_ids=list(range(NCORES)))
    return np.concatenate([r["out"] for r in res.results], axis=0)
